# revision 1
# baseline (speedup 1.0000x reference)
"""Causal single-head attention (B=4, T=2048, C=1024, fp32) on 8 TRN2 NeuronCores.

Sharding: core c -> (batch b = c//2, T-half h = c%2). Each core computes
q = x_q @ Wq.T for its 1024 query rows, k/v for the full 2048 rows of its batch,
then causal attention for its queries. All inputs are pre-transposed on the host
so the device never transposes anything:

  qT[d, q]  = WqT_chunk.T @ xqT          (lhsT = WqT block [c,d], rhs = xqT [c,q])
  kT[d, t]  = WkT_chunk.T @ xkvT         (spilled to internal DRAM, streamed back)
  v [t, d]  = xkvT_chunk.T @ WvT         (natural layout for the av matmul)
  sT[k, q]  = kT_block.T  @ qT           (qk transposed: softmax reductions on PE)
  expT      = Exp(sT * C**-0.5)          (unstabilized: max qk ~ 8.3, exp <= 4100)
  expT     *= (qpos >= kpos)             (causal mask built on-device)
  denom[q]  = expT_chunk.T @ ones        (PSUM-accumulated over k chunks)
  av[q, d]  = expT_chunk.T @ v_chunk     (PSUM-accumulated over k chunks)
  out       = av * (1/denom)             (fused into PSUM eviction)

All matmuls run as float32r (full fp32 data, 1 cycle/row when N >= 256).
"""

import numpy as np

B, T, C = 4, 2048, 1024
NCORES = 8
TQ = T // 2          # queries per core
P = 128              # partitions
F32R_N_MIN = 256

TRACE = False        # set True from test.py to get NTFF profile + exec_time_ns
LAST_RESULTS = None  # BassKernelResults of the last run (for test.py)

_COMPILED = None


def _build_program():
    import concourse.bacc as bacc
    import concourse.mybir as mybir
    import concourse.tile as tile

    f32 = mybir.dt.float32
    f32r = mybir.dt.float32r
    SCALE = float(C) ** -0.5

    nc = bacc.Bacc("TRN2", target_bir_lowering=False, debug=False,
                   num_devices=NCORES)

    xqT_d = nc.dram_tensor("xqT", [C, TQ], f32r, kind="ExternalInput").ap()
    xkvT_d = nc.dram_tensor("xkvT", [C, T], f32r, kind="ExternalInput").ap()
    WqT_d = nc.dram_tensor("WqT", [C, C], f32r, kind="ExternalInput").ap()
    WkT_d = nc.dram_tensor("WkT", [C, C], f32r, kind="ExternalInput").ap()
    WvT_d = nc.dram_tensor("WvT", [C, C], f32r, kind="ExternalInput").ap()
    qposb_d = nc.dram_tensor("qposb", [P, TQ], f32, kind="ExternalInput").ap()
    kpos_d = nc.dram_tensor("kpos", [P, T // P], f32, kind="ExternalInput").ap()
    out_d = nc.dram_tensor("out", [TQ, C], f32, kind="ExternalOutput").ap()
    # kT spill buffer (per-core scratch DRAM)
    kTd = nc.dram_tensor("kTspill", [C, T], f32r, kind="Internal").ap()

    CC = C // P   # 8 contraction chunks
    KT = T // P   # 16 key tiles
    QT8 = TQ // P  # 8 query tiles

    with tile.TileContext(nc, pool_alloc_mode="queue") as tc:
        with tc.tile_pool(name="persist", bufs=1) as persist:
            # v resident in SBUF: [t-part, t-chunk, d] = [128, 16, 1024] fp32
            v_sb = persist.tile([P, KT, C], f32r, tag="v_sb")
            qT_sb = persist.tile([P, CC, TQ], f32r, tag="qT_sb")

            # ---------------- Phase A1: kT (-> DRAM) and v (-> SBUF) ------
            with tc.tile_pool(name="a1", bufs=1) as a1, \
                 tc.tile_pool(name="xh_pool", bufs=2) as xh_pool, \
                 tc.tile_pool(name="wk_pool", bufs=4) as wk_pool, \
                 tc.tile_pool(name="kstg_pool", bufs=4) as kstg_pool, \
                 tc.tile_pool(name="pk", bufs=2, space="PSUM") as pk_pool, \
                 tc.tile_pool(name="pv", bufs=2, space="PSUM") as pv_pool:
                # full WvT resident: [c-part, c-chunk, d]
                wvt = a1.tile([P, CC, C], f32r, tag="wvt")
                for cc in range(CC):
                    nc.sync.dma_start(
                        wvt[:, cc, :], WvT_d[cc * P:(cc + 1) * P, :])

                for th in range(2):
                    xh = xh_pool.tile([P, CC, TQ], f32r, tag="xh")
                    for cc in range(CC):
                        nc.sync.dma_start(
                            xh[:, cc, :],
                            xkvT_d[cc * P:(cc + 1) * P,
                                   th * TQ:(th + 1) * TQ])

                    # kT: out [d-tile 128, t 512] accumulated over c chunks
                    for dt in range(CC):
                        pks = [pk_pool.tile([P, 512], f32, tag=f"pk{i}", name=f"pk{i}")
                               for i in range(2)]
                        for cc in range(CC):
                            wk = wk_pool.tile([P, P], f32r, tag="wk")
                            nc.sync.dma_start(
                                wk[:],
                                WkT_d[cc * P:(cc + 1) * P,
                                      dt * P:(dt + 1) * P])
                            for tq in range(2):
                                nc.tensor.matmul(
                                    pks[tq][:],
                                    wk[:],
                                    xh[:, cc, tq * 512:(tq + 1) * 512]
                                    ,
                                    start=(cc == 0), stop=(cc == CC - 1))
                        for tq in range(2):
                            ks = kstg_pool.tile([P, 512], f32r, tag="ks")
                            nc.vector.tensor_copy(ks[:], pks[tq][:])
                            nc.sync.dma_start(
                                kTd[dt * P:(dt + 1) * P,
                                    th * TQ + tq * 512: th * TQ + (tq + 1) * 512],
                                ks[:])

                    # v: out [t-tile 128, d 512] accumulated over c chunks
                    for tt in range(QT8):
                        pvs = [pv_pool.tile([P, 512], f32, tag=f"pv{i}", name=f"pv{i}")
                               for i in range(2)]
                        for cc in range(CC):
                            for dh in range(2):
                                nc.tensor.matmul(
                                    pvs[dh][:],
                                    xh[:, cc, tt * P:(tt + 1) * P]
                                    ,
                                    wvt[:, cc, dh * 512:(dh + 1) * 512]
                                    ,
                                    start=(cc == 0), stop=(cc == CC - 1))
                        for dh in range(2):
                            nc.vector.tensor_copy(
                                v_sb[:, th * QT8 + tt,
                                     dh * 512:(dh + 1) * 512],
                                pvs[dh][:])

            # ---------------- Phase A2: qT (-> SBUF) ----------------------
            with tc.tile_pool(name="a2", bufs=1) as a2, \
                 tc.tile_pool(name="wq_pool", bufs=4) as wq_pool, \
                 tc.tile_pool(name="pq", bufs=2, space="PSUM") as pq_pool:
                xq = a2.tile([P, CC, TQ], f32r, tag="xq")
                for cc in range(CC):
                    nc.sync.dma_start(
                        xq[:, cc, :], xqT_d[cc * P:(cc + 1) * P, :])
                for dt in range(CC):
                    pqs = [pq_pool.tile([P, 512], f32, tag=f"pq{i}", name=f"pq{i}")
                           for i in range(2)]
                    for cc in range(CC):
                        wq = wq_pool.tile([P, P], f32r, tag="wq")
                        nc.sync.dma_start(
                            wq[:],
                            WqT_d[cc * P:(cc + 1) * P, dt * P:(dt + 1) * P])
                        for qh in range(2):
                            nc.tensor.matmul(
                                pqs[qh][:],
                                wq[:],
                                xq[:, cc, qh * 512:(qh + 1) * 512]
                                ,
                                start=(cc == 0), stop=(cc == CC - 1))
                    for qh in range(2):
                        nc.vector.tensor_copy(
                            qT_sb[:, dt, qh * 512:(qh + 1) * 512], pqs[qh][:])

            # ---------------- Phase B: attention --------------------------
            with tc.tile_pool(name="battn", bufs=1) as battn:
                expT = battn.tile([P, KT, TQ], f32r, tag="expT")
                qposb = battn.tile([P, TQ], f32, tag="qposb")
                kpos = battn.tile([P, KT], f32, tag="kpos")
                ones_f = battn.tile([P, 8], f32, tag="ones_f")
                ones = battn.tile([P, 8], f32r, tag="ones")
                nc.sync.dma_start(qposb[:], qposb_d[:, :])
                nc.sync.dma_start(kpos[:], kpos_d[:, :])
                nc.vector.memset(ones_f[:], 1.0)
                nc.vector.tensor_copy(ones[:], ones_f[:])

                # sT + exp + mask, key-tile major
                with tc.tile_pool(name="ktile_pool", bufs=3) as ktile_pool, \
                     tc.tile_pool(name="msk_pool", bufs=4) as msk_pool, \
                     tc.tile_pool(name="ps", bufs=2, space="PSUM") as ps_pool:
                    for kt in range(KT):
                        ktile = ktile_pool.tile([P, CC, P], f32r, tag="ktile")
                        nc.sync.dma_start(
                            ktile[:],
                            kTd[:, kt * P:(kt + 1) * P]
                            .rearrange("(dc p) k -> p dc k", p=P))
                        pss = [ps_pool.tile([P, 512], f32, tag=f"ps{i}", name=f"ps{i}")
                               for i in range(2)]
                        for dc in range(CC):
                            for qh in range(2):
                                nc.tensor.matmul(
                                    pss[qh][:],
                                    ktile[:, dc, :],
                                    qT_sb[:, dc, qh * 512:(qh + 1) * 512]
                                    ,
                                    start=(dc == 0), stop=(dc == CC - 1))
                        for qh in range(2):
                            sl = slice(qh * 512, (qh + 1) * 512)
                            msk = msk_pool.tile([P, 512], f32, tag="msk")
                            nc.vector.tensor_scalar(
                                msk[:], qposb[:, sl], kpos[:, kt:kt + 1],
                                None, op0=mybir.AluOpType.is_ge)
                            nc.scalar.activation(
                                expT[:, kt, sl], pss[qh][:],
                                mybir.ActivationFunctionType.Exp,
                                bias=0.0, scale=SCALE)
                            nc.vector.tensor_tensor(
                                expT[:, kt, sl], expT[:, kt, sl], msk[:],
                                op=mybir.AluOpType.mult)

                # av + denom + normalize, query-tile major
                with tc.tile_pool(name="out_pool", bufs=4) as out_pool, \
                     tc.tile_pool(name="rec_pool", bufs=2) as rec_pool, \
                     tc.tile_pool(name="pav", bufs=2, space="PSUM") as pav_pool, \
                     tc.tile_pool(name="pden", bufs=2, space="PSUM") as pden_pool:
                    for qt in range(QT8):
                        pavs = [pav_pool.tile([P, 512], f32, tag=f"pav{i}", name=f"pav{i}")
                                for i in range(2)]
                        pden = pden_pool.tile([P, 8], f32, tag="pden")
                        for kc in range(KT):
                            lhs = expT[:, kc, qt * P:(qt + 1) * P] \
                                
                            for dh in range(2):
                                nc.tensor.matmul(
                                    pavs[dh][:], lhs,
                                    v_sb[:, kc, dh * 512:(dh + 1) * 512]
                                    ,
                                    start=(kc == 0), stop=(kc == KT - 1))
                            nc.tensor.matmul(
                                pden[:], lhs, ones[:],
                                start=(kc == 0), stop=(kc == KT - 1))

                        rec = rec_pool.tile([P, 1], f32, tag="rec")
                        nc.vector.reciprocal(rec[:], pden[:, 0:1])
                        for dh in range(2):
                            ot = out_pool.tile([P, 512], f32, tag="ot")
                            nc.vector.tensor_scalar(
                                ot[:], pavs[dh][:], rec[:], None,
                                op0=mybir.AluOpType.mult)
                            nc.sync.dma_start(
                                out_d[qt * P:(qt + 1) * P,
                                      dh * 512:(dh + 1) * 512],
                                ot[:])

    nc.compile()
    return nc


def _get_compiled():
    global _COMPILED
    if _COMPILED is None:
        _COMPILED = _build_program()
    return _COMPILED


def _tf32_round(a):
    """Round fp32 to TF32 (10-bit mantissa), round-to-nearest-even."""
    u = a.view(np.uint32)
    r = ((u >> 13) + ((u >> 12) & 1)) << 13  # RNE-ish (ties up); fine here
    return r.astype(np.uint32).view(np.float32)


def _enable_ldw_opt():
    """walrus elides redundant back-to-back LDWEIGHTS with ldw-opt on; the
    repo default pins it off. Half our weight loads are consecutive dupes."""
    import concourse.bass_utils as _bu
    if getattr(_bu, "_ldw_patched", False):
        return
    orig = _bu.run_command

    def patched(argv, **kw):
        argv = ["--enable-ldw-opt=true" if a == "--enable-ldw-opt=false"
                else a for a in argv]
        return orig(argv, **kw)

    _bu.run_command = patched
    _bu._ldw_patched = True


def kernel(x, Wq, Wk, Wv):
    global LAST_RESULTS
    _enable_ldw_opt()
    from concourse.bass_utils import run_bass_kernel_spmd

    x = _tf32_round(np.ascontiguousarray(np.asarray(x, dtype=np.float32)))
    WqT = _tf32_round(np.ascontiguousarray(np.asarray(Wq, dtype=np.float32).T))
    WkT = _tf32_round(np.ascontiguousarray(np.asarray(Wk, dtype=np.float32).T))
    WvT = _tf32_round(np.ascontiguousarray(np.asarray(Wv, dtype=np.float32).T))

    kpos = (np.arange(T // P)[None, :] * P
            + np.arange(P)[:, None]).astype(np.float32)

    in_maps = []
    for c in range(NCORES):
        b, h = divmod(c, 2)
        xb_T = np.ascontiguousarray(x[b].T)            # [C, T]
        xqT = np.ascontiguousarray(xb_T[:, h * TQ:(h + 1) * TQ])
        qpos = np.arange(h * TQ, (h + 1) * TQ, dtype=np.float32)
        qposb = np.ascontiguousarray(
            np.broadcast_to(qpos[None, :], (P, TQ)))
        in_maps.append({
            "xqT": xqT, "xkvT": xb_T,
            "WqT": WqT, "WkT": WkT, "WvT": WvT,
            "qposb": qposb, "kpos": kpos,
        })

    nc = _get_compiled()
    res = run_bass_kernel_spmd(nc, in_maps, core_ids=list(range(NCORES)),
                               trace=TRACE)
    LAST_RESULTS = res

    out = np.empty((B, T, C), dtype=np.float32)
    for c in range(NCORES):
        b, h = divmod(c, 2)
        out[b, h * TQ:(h + 1) * TQ, :] = res.results[c]["out"]
    return out



# revision 4
# speedup vs baseline: 1.0085x; 1.0085x over previous
"""Causal single-head attention (B=4, T=2048, C=1024, fp32) on 8 TRN2 NeuronCores.

Sharding: cores 2b and 2b+1 pair up on batch b. Within a pair (rank r = core%2):

  - query tiles (128 rows each) are interleaved even/odd: rank r owns global
    q tiles {2s + r : s in 0..7}. This balances causal work AND makes the
    program rank-independent (one NEFF runs SPMD on all 8 cores; causality
    beyond the computed tile set is enforced by mask *data*, not control flow).
  - k/v projections are split: rank r projects keys [1024r, 1024r+1024), then
    the halves are exchanged with a pairwise AllGather (HBM bounce buffers),
    eliminating the duplicated k/v projection of the all-local scheme.

Per-core schedule (identical on every core):
  K proj (my key half)   -> cc_k_in  --AllGather--> k_sb [128, 2, 8, 1024]
  V proj (my key half)   -> cc_v_in  --AllGather--> v_sb (bf16)
  Q proj (my q tiles)    -> qT_sb
  scores chunk c (512 q): kt in [0, N_SC[c]) ; exp(scale*s)*mask -> bf16
  AV slot s (128 q):      kc in [0, N_AV[s]) accumulated in PSUM + denom
  out = av * (1/denom)

N_SC = [8, 16] and N_AV[s] = 2s+2 cover the causal needs of BOTH ranks'
tile sets (the max over the pair), so the instruction streams are identical;
the fully/partially masked remainder is zeroed by the is_ge mask.

Matmuls: projections and scores in float32r (full fp32, 1 cycle/row at
N>=256); exp weights and v in bfloat16 (error ~0.4% << 2e-2 gate; halves
SBUF and AllGather traffic for the AV path). exp is unstabilized as in the
baseline (max qk ~ 8.3 -> exp <= 4100).
"""

import numpy as np

B, T, C = 4, 2048, 1024
NCORES = 8
P = 128              # partitions
NQ = T // 2          # local queries / local keys per core (1024)
CC = C // P          # 8 contraction chunks
NKT = T // P         # 16 global key tiles
N_SC = [8, 16]       # key tiles per 512-q scores chunk
N_AV = [2, 4, 6, 8, 10, 12, 14, 16]   # key tiles per 128-q AV slot
RG = [[0, 1], [2, 3], [4, 5], [6, 7]]

TRACE = False        # set True from test.py to get NTFF profile + exec_time_ns
LAST_RESULTS = None  # BassKernelResults of the last run (for test.py)

_COMPILED = None


def _build_program():
    import concourse.bacc as bacc
    import concourse.mybir as mybir
    import concourse.tile as tile

    f32 = mybir.dt.float32
    f32r = mybir.dt.float32r
    bf16 = mybir.dt.bfloat16
    SCALE = float(C) ** -0.5

    nc = bacc.Bacc("TRN2", target_bir_lowering=False, debug=False,
                   num_devices=NCORES)

    xqT_d = nc.dram_tensor("xqT", [C, NQ], f32r, kind="ExternalInput").ap()
    xkvT_d = nc.dram_tensor("xkvT", [C, NQ], f32r, kind="ExternalInput").ap()
    WqT_d = nc.dram_tensor("WqT", [C, C], f32r, kind="ExternalInput").ap()
    WkT_d = nc.dram_tensor("WkT", [C, C], f32r, kind="ExternalInput").ap()
    WvT_d = nc.dram_tensor("WvT", [C, C], f32r, kind="ExternalInput").ap()
    qposb_d = nc.dram_tensor("qposb", [P, NQ], f32, kind="ExternalInput").ap()
    kpos_d = nc.dram_tensor("kpos", [P, NKT], f32, kind="ExternalInput").ap()
    out_d = nc.dram_tensor("out", [NQ, C], f32, kind="ExternalOutput").ap()

    with tile.TileContext(nc, pool_alloc_mode="queue") as tc:
        with tc.tile_pool(name="dram", bufs=1, space="DRAM") as dpool, \
             tc.tile_pool(name="persist", bufs=1) as persist:
            # collective bounce buffers: k in [p, dt, keys] layout so the
            # gathered halves DMA straight into k_sb with big descriptors
            cc_k_in = dpool.tile([P, CC, NQ], f32r, tag="cc_k_in")
            cc_k_out = dpool.tile([2, P, CC, NQ], f32r, tag="cc_k_out")
            cc_v_in = dpool.tile([NQ, C], bf16, tag="cc_v_in")
            cc_v_out = dpool.tile([2, NQ, C], bf16, tag="cc_v_out")

            # k_sb[:, p, dc, k]: stationary blocks for scores (d on partitions)
            k_sb = persist.tile([P, 2, CC, NQ], f32r, tag="k_sb")
            # v_sb[:, kc, d]: AV rhs (key pos on partitions), bf16
            v_sb = persist.tile([P, NKT, C], bf16, tag="v_sb")
            # qT_sb[:, dc, q]: scores rhs (d on partitions)
            qT_sb = persist.tile([P, CC, NQ], f32r, tag="qT_sb")

            # ---------------- K proj + AG_k, V proj + AG_v ----------------
            with tc.tile_pool(name="kvp", bufs=1) as kvp, \
                 tc.tile_pool(name="wk_pool", bufs=4) as wk_pool, \
                 tc.tile_pool(name="kstg_pool", bufs=3) as kstg_pool, \
                 tc.tile_pool(name="vstg_pool", bufs=3) as vstg_pool, \
                 tc.tile_pool(name="pk", bufs=2, space="PSUM") as pk_pool, \
                 tc.tile_pool(name="pv", bufs=2, space="PSUM") as pv_pool:
                xkv = kvp.tile([P, CC, NQ], f32r, tag="xkv")
                wvt = kvp.tile([P, CC, C], f32r, tag="wvt")
                for cc in range(CC):
                    nc.sync.dma_start(
                        xkv[:, cc, :], xkvT_d[cc * P:(cc + 1) * P, :])
                for cc in range(CC):
                    nc.sync.dma_start(
                        wvt[:, cc, :], WvT_d[cc * P:(cc + 1) * P, :])

                # kT: out [d-tile 128, keys 512] accumulated over c chunks
                for dt in range(CC):
                    pks = [pk_pool.tile([P, 512], f32, tag=f"pk{i}",
                                        name=f"pk{i}") for i in range(2)]
                    for cc in range(CC):
                        wk = wk_pool.tile([P, P], f32r, tag="wk", name="wk")
                        nc.sync.dma_start(
                            wk[:],
                            WkT_d[cc * P:(cc + 1) * P, dt * P:(dt + 1) * P])
                        for h in range(2):
                            nc.tensor.matmul(
                                pks[h][:], wk[:],
                                xkv[:, cc, h * 512:(h + 1) * 512],
                                start=(cc == 0), stop=(cc == CC - 1))
                    for h in range(2):
                        ks = kstg_pool.tile([P, 512], f32r, tag="ks",
                                            name="ks")
                        nc.vector.tensor_copy(ks[:], pks[h][:])
                        nc.sync.dma_start(
                            cc_k_in[:, dt, h * 512:(h + 1) * 512], ks[:])

                nc.gpsimd.collective_compute(
                    "AllGather", mybir.AluOpType.bypass, replica_groups=RG,
                    ins=[cc_k_in.opt()], outs=[cc_k_out.opt()])
                for p in range(2):
                    nc.sync.dma_start(k_sb[:, p], cc_k_out[p])

                # v: out [key-tile 128, d 512] accumulated over c chunks
                for kt in range(CC):
                    pvs = [pv_pool.tile([P, 512], f32, tag=f"pv{i}",
                                        name=f"pv{i}") for i in range(2)]
                    for cc in range(CC):
                        for dh in range(2):
                            nc.tensor.matmul(
                                pvs[dh][:],
                                xkv[:, cc, kt * P:(kt + 1) * P],
                                wvt[:, cc, dh * 512:(dh + 1) * 512],
                                start=(cc == 0), stop=(cc == CC - 1))
                    for dh in range(2):
                        vs = vstg_pool.tile([P, 512], bf16, tag="vs",
                                            name="vs")
                        nc.vector.tensor_copy(vs[:], pvs[dh][:])
                        nc.sync.dma_start(
                            cc_v_in[kt * P:(kt + 1) * P,
                                    dh * 512:(dh + 1) * 512], vs[:])

                nc.gpsimd.collective_compute(
                    "AllGather", mybir.AluOpType.bypass, replica_groups=RG,
                    ins=[cc_v_in.opt()], outs=[cc_v_out.opt()])
                for p in range(2):
                    for kt in range(CC):
                        nc.sync.dma_start(
                            v_sb[:, p * CC + kt, :],
                            cc_v_out[p, kt * P:(kt + 1) * P, :])

            # ---------------- Q proj (local tiles only) -------------------
            with tc.tile_pool(name="qp", bufs=1) as qp, \
                 tc.tile_pool(name="wq_pool", bufs=4) as wq_pool, \
                 tc.tile_pool(name="pq", bufs=2, space="PSUM") as pq_pool:
                xq = qp.tile([P, CC, NQ], f32r, tag="xq")
                for cc in range(CC):
                    nc.sync.dma_start(
                        xq[:, cc, :], xqT_d[cc * P:(cc + 1) * P, :])
                for dt in range(CC):
                    pqs = [pq_pool.tile([P, 512], f32, tag=f"pq{i}",
                                        name=f"pq{i}") for i in range(2)]
                    for cc in range(CC):
                        wq = wq_pool.tile([P, P], f32r, tag="wq", name="wq")
                        nc.sync.dma_start(
                            wq[:],
                            WqT_d[cc * P:(cc + 1) * P, dt * P:(dt + 1) * P])
                        for h in range(2):
                            nc.tensor.matmul(
                                pqs[h][:], wq[:],
                                xq[:, cc, h * 512:(h + 1) * 512],
                                start=(cc == 0), stop=(cc == CC - 1))
                    for h in range(2):
                        nc.vector.tensor_copy(
                            qT_sb[:, dt, h * 512:(h + 1) * 512], pqs[h][:])

            # ---------------- attention -----------------------------------
            with tc.tile_pool(name="attn", bufs=1) as attn, \
                 tc.tile_pool(name="msk_pool", bufs=4) as msk_pool, \
                 tc.tile_pool(name="exp_pool", bufs=1) as exp_pool, \
                 tc.tile_pool(name="out_pool", bufs=4) as out_pool, \
                 tc.tile_pool(name="rec_pool", bufs=2) as rec_pool, \
                 tc.tile_pool(name="ps", bufs=2, space="PSUM") as ps_pool, \
                 tc.tile_pool(name="pav", bufs=2, space="PSUM") as pav_pool, \
                 tc.tile_pool(name="pden", bufs=2, space="PSUM") as pden_pool:
                qposb = attn.tile([P, NQ], f32, tag="qposb")
                kpos = attn.tile([P, NKT], f32, tag="kpos")
                ones_f = attn.tile([P, 8], f32, tag="ones_f")
                ones = attn.tile([P, 8], bf16, tag="ones")
                nc.sync.dma_start(qposb[:], qposb_d[:, :])
                nc.sync.dma_start(kpos[:], kpos_d[:, :])
                nc.vector.memset(ones_f[:], 1.0)
                nc.vector.tensor_copy(ones[:], ones_f[:])

                for c in range(2):
                    sl = slice(c * 512, (c + 1) * 512)
                    ex = exp_pool.tile([P, N_SC[c], 512], bf16,
                                       tag=f"exp{c}", name=f"exp{c}")
                    for kt in range(N_SC[c]):
                        ps = ps_pool.tile([P, 512], f32, tag="ps", name="ps")
                        for dc in range(CC):
                            nc.tensor.matmul(
                                ps[:],
                                k_sb[:, kt // CC, dc,
                                     (kt % CC) * P:(kt % CC + 1) * P],
                                qT_sb[:, dc, sl],
                                start=(dc == 0), stop=(dc == CC - 1))
                        msk = msk_pool.tile([P, 512], bf16, tag="msk",
                                            name="msk")
                        nc.vector.tensor_scalar(
                            msk[:], qposb[:, sl], kpos[:, kt:kt + 1],
                            None, op0=mybir.AluOpType.is_ge)
                        nc.scalar.activation(
                            ex[:, kt, :], ps[:],
                            mybir.ActivationFunctionType.Exp,
                            bias=0.0, scale=SCALE)
                        nc.vector.tensor_tensor(
                            ex[:, kt, :], ex[:, kt, :], msk[:],
                            op=mybir.AluOpType.mult)

                    for s in range(4 * c, 4 * c + 4):
                        pavs = [pav_pool.tile([P, 512], f32, tag=f"pav{i}",
                                              name=f"pav{i}")
                                for i in range(2)]
                        pden = pden_pool.tile([P, 8], f32, tag="pden",
                                              name="pden")
                        n = N_AV[s]
                        so = (s - 4 * c) * P
                        for kc in range(n):
                            lhs = ex[:, kc, so:so + P]
                            for dh in range(2):
                                nc.tensor.matmul(
                                    pavs[dh][:], lhs,
                                    v_sb[:, kc, dh * 512:(dh + 1) * 512],
                                    start=(kc == 0), stop=(kc == n - 1))
                            nc.tensor.matmul(
                                pden[:], lhs, ones[:],
                                start=(kc == 0), stop=(kc == n - 1))

                        rec = rec_pool.tile([P, 1], f32, tag="rec",
                                            name="rec")
                        nc.vector.reciprocal(rec[:], pden[:, 0:1])
                        for dh in range(2):
                            ot = out_pool.tile([P, 512], f32, tag="ot",
                                               name="ot")
                            nc.vector.tensor_scalar(
                                ot[:], pavs[dh][:], rec[:], None,
                                op0=mybir.AluOpType.mult)
                            nc.sync.dma_start(
                                out_d[s * P:(s + 1) * P,
                                      dh * 512:(dh + 1) * 512],
                                ot[:])

    nc.compile()
    return nc


def _get_compiled():
    global _COMPILED
    if _COMPILED is None:
        _COMPILED = _build_program()
    return _COMPILED


def _tf32_round(a):
    """Round fp32 to TF32 (10-bit mantissa), round-to-nearest-even."""
    u = a.view(np.uint32)
    r = ((u >> 13) + ((u >> 12) & 1)) << 13  # RNE-ish (ties up); fine here
    return r.astype(np.uint32).view(np.float32)


def _enable_ldw_opt():
    """walrus elides redundant back-to-back LDWEIGHTS with ldw-opt on; the
    repo default pins it off. Half our weight loads are consecutive dupes."""
    import concourse.bass_utils as _bu
    if getattr(_bu, "_ldw_patched", False):
        return
    orig = _bu.run_command

    def patched(argv, **kw):
        argv = ["--enable-ldw-opt=true" if a == "--enable-ldw-opt=false"
                else a for a in argv]
        return orig(argv, **kw)

    _bu.run_command = patched
    _bu._ldw_patched = True


def kernel(x, Wq, Wk, Wv):
    global LAST_RESULTS
    # NOTE: ldw-opt stays off — walrus rejects bf16 stationary (exp tiles)
    # under --enable-ldw-opt=true ("InstLdweights is not compatible").
    from concourse.bass_utils import run_bass_kernel_spmd

    x = _tf32_round(np.ascontiguousarray(np.asarray(x, dtype=np.float32)))
    WqT = _tf32_round(np.ascontiguousarray(np.asarray(Wq, dtype=np.float32).T))
    WkT = _tf32_round(np.ascontiguousarray(np.asarray(Wk, dtype=np.float32).T))
    WvT = _tf32_round(np.ascontiguousarray(np.asarray(Wv, dtype=np.float32).T))

    kpos = (np.arange(NKT)[None, :] * P
            + np.arange(P)[:, None]).astype(np.float32)

    in_maps = []
    for core in range(NCORES):
        b, r = divmod(core, 2)
        xb_T = np.ascontiguousarray(x[b].T)            # [C, T]
        qcols = np.concatenate(
            [np.arange((2 * s + r) * P, (2 * s + r + 1) * P)
             for s in range(8)])
        xqT = np.ascontiguousarray(xb_T[:, qcols])
        xkvT = np.ascontiguousarray(xb_T[:, r * NQ:(r + 1) * NQ])
        qposb = np.ascontiguousarray(np.broadcast_to(
            qcols.astype(np.float32)[None, :], (P, NQ)))
        in_maps.append({
            "xqT": xqT, "xkvT": xkvT,
            "WqT": WqT, "WkT": WkT, "WvT": WvT,
            "qposb": qposb, "kpos": kpos,
        })

    nc = _get_compiled()
    res = run_bass_kernel_spmd(nc, in_maps, core_ids=list(range(NCORES)),
                               trace=TRACE)
    LAST_RESULTS = res

    out = np.empty((B, T, C), dtype=np.float32)
    for core in range(NCORES):
        b, r = divmod(core, 2)
        oc = res.results[core]["out"]                  # [NQ, C] local order
        for s in range(8):
            out[b, (2 * s + r) * P:(2 * s + r + 1) * P, :] = \
                oc[s * P:(s + 1) * P, :]
    return out


# revision 7
# speedup vs baseline: 1.0320x; 1.0233x over previous
"""Causal single-head attention (B=4, T=2048, C=1024, fp32) on 8 TRN2 NeuronCores.

Sharding: cores 2b and 2b+1 pair up on batch b. Within a pair (rank r = core%2):

  - query tiles (128 rows) interleave even/odd: rank r owns global q tiles
    {2s + r : s in 0..7}. Balances causal work AND keeps the program
    rank-independent (one NEFF runs SPMD on all 8 cores; causality beyond
    the computed tile set is enforced by mask *data*, not control flow).
  - k/v projections split: rank r projects keys [1024r, 1024r+1024), halves
    exchanged with pairwise AllGathers, eliminating duplicated k/v work.

Collectives: each 2MB AllGather is split into 4 x 512KB calls (mesh-algo
regime, <1MB; a single 2-4MB call lands in the slow ring regime: measured
107us for 2MB vs ~5us floor for mesh) and fired progressively during the
projection loops so they hide under compute. Wire dtype is bf16; SBUF-side
tensors are float32r (walrus rejects bf16 matmul stationaries under
--enable-ldw-opt, and ldw-opt is worth ~150ns/matmul of LDWEIGHTS overlap).

Per-core schedule (identical on every core):
  K proj -> cc_k_in[j] (bf16) --AG--> k_sb (kt 0..7 resident f32r)
  V proj -> cc_v_in[j] (bf16) --AG--> v_sb f32r
  Q proj -> qT_sb f32r
  scores chunk c (512 q): kt in [0, N_SC[c]); kt>=8 streamed from cc_k_out
  AV slot s (128 q): kc in [0, N_AV[s]) PSUM-accumulated + denom matmul
  out = av * (1/denom)

N_SC = [8, 16], N_AV[s] = 2s+2 cover the causal needs of BOTH ranks' tile
sets (max over the pair), so instruction streams are identical; the masked
remainder contributes exact zeros.
"""

import numpy as np

B, T, C = 4, 2048, 1024
NCORES = 8
P = 128              # partitions
NQ = T // 2          # local queries / local keys per core (1024)
CC = C // P          # 8 contraction chunks
NKT = T // P         # 16 global key tiles
N_SC = [8, 16]       # key tiles per 512-q scores chunk
N_AV = [2, 4, 6, 8, 10, 12, 14, 16]   # key tiles per 128-q AV slot
RG = [[0, 1], [2, 3], [4, 5], [6, 7]]

TRACE = False        # set True from test.py to get NTFF profile + exec_time_ns
LAST_RESULTS = None  # BassKernelResults of the last run (for test.py)

_COMPILED = None


def _build_program():
    import concourse.bacc as bacc
    import concourse.mybir as mybir
    import concourse.tile as tile

    f32 = mybir.dt.float32
    f32r = mybir.dt.float32r
    bf16 = mybir.dt.bfloat16
    SCALE = float(C) ** -0.5

    nc = bacc.Bacc("TRN2", target_bir_lowering=False, debug=False,
                   num_devices=NCORES)

    xqT_d = nc.dram_tensor("xqT", [C, NQ], f32r, kind="ExternalInput").ap()
    xkvT_d = nc.dram_tensor("xkvT", [C, NQ], f32r, kind="ExternalInput").ap()
    WqT_d = nc.dram_tensor("WqT", [C, C], f32r, kind="ExternalInput").ap()
    WkT_d = nc.dram_tensor("WkT", [C, C], f32r, kind="ExternalInput").ap()
    WvT_d = nc.dram_tensor("WvT", [C, C], f32r, kind="ExternalInput").ap()
    qposb_d = nc.dram_tensor("qposb", [P, NQ], f32, kind="ExternalInput").ap()
    kpos_d = nc.dram_tensor("kpos", [P, NKT], f32, kind="ExternalInput").ap()
    out_d = nc.dram_tensor("out", [NQ, C], f32, kind="ExternalOutput").ap()

    with tile.TileContext(nc, pool_alloc_mode="queue") as tc:
        with tc.tile_pool(name="dram", bufs=1, space="DRAM") as dpool, \
             tc.tile_pool(name="persist", bufs=1) as persist:
            # 512KB bounce buffers: k chunk j = d-tiles {2j, 2j+1} x my keys;
            # v chunk j = my key slots {2j, 2j+1} x full d
            cc_k_in = [dpool.tile([P, 2, NQ], bf16, tag=f"cc_k_in{j}",
                                  name=f"cc_k_in{j}") for j in range(4)]
            cc_k_out = [dpool.tile([2, P, 2, NQ], bf16, tag=f"cc_k_out{j}",
                                   name=f"cc_k_out{j}") for j in range(4)]
            cc_v_in = [dpool.tile([2 * P, C], bf16, tag=f"cc_v_in{j}",
                                  name=f"cc_v_in{j}") for j in range(4)]
            cc_v_out = [dpool.tile([2, 2 * P, C], bf16, tag=f"cc_v_out{j}",
                                   name=f"cc_v_out{j}") for j in range(4)]

            # k_sb[:, dc, k]: kt 0..7 (pair-rank 0 keys) f32r stationaries
            k_sb = persist.tile([P, CC, NQ], f32r, tag="k_sb")
            # v_sb[:, kc, d]: all 16 key tiles
            v_sb = persist.tile([P, NKT, C], f32r, tag="v_sb")
            # qT_sb[:, dc, q]: my 1024 queries (local col order)
            qT_sb = persist.tile([P, CC, NQ], f32r, tag="qT_sb")

            # ---------------- K proj + chunked AG_k -----------------------
            with tc.tile_pool(name="kvp", bufs=1) as kvp, \
                 tc.tile_pool(name="wvh_pool", bufs=1) as wvh_pool, \
                 tc.tile_pool(name="wk_pool", bufs=4) as wk_pool, \
                 tc.tile_pool(name="kstg_pool", bufs=3) as kstg_pool, \
                 tc.tile_pool(name="vstg_pool", bufs=3) as vstg_pool, \
                 tc.tile_pool(name="kbf_pool", bufs=2) as kbf_pool, \
                 tc.tile_pool(name="vbf_pool", bufs=3) as vbf_pool, \
                 tc.tile_pool(name="pk", bufs=2, space="PSUM") as pk_pool, \
                 tc.tile_pool(name="pv", bufs=3, space="PSUM") as pv_pool:
                xkv = kvp.tile([P, CC, NQ], f32r, tag="xkv")
                for cc in range(CC):
                    nc.sync.dma_start(
                        xkv[:, cc, :], xkvT_d[cc * P:(cc + 1) * P, :])

                for dt in range(CC):
                    pks = [pk_pool.tile([P, 512], f32, tag=f"pk{i}",
                                        name=f"pk{i}") for i in range(2)]
                    for cc in range(CC):
                        wk = wk_pool.tile([P, P], f32r, tag="wk", name="wk")
                        nc.sync.dma_start(
                            wk[:],
                            WkT_d[cc * P:(cc + 1) * P, dt * P:(dt + 1) * P])
                        for h in range(2):
                            nc.tensor.matmul(
                                pks[h][:], wk[:],
                                xkv[:, cc, h * 512:(h + 1) * 512],
                                start=(cc == 0), stop=(cc == CC - 1))
                    for h in range(2):
                        ks = kstg_pool.tile([P, 512], bf16, tag="ks",
                                            name="ks")
                        nc.vector.tensor_copy(ks[:], pks[h][:])
                        nc.sync.dma_start(
                            cc_k_in[dt // 2][:, dt % 2,
                                             h * 512:(h + 1) * 512], ks[:])
                    if dt % 2 == 1:
                        j = dt // 2
                        nc.gpsimd.collective_compute(
                            "AllGather", mybir.AluOpType.bypass,
                            replica_groups=RG,
                            ins=[cc_k_in[j].opt()], outs=[cc_k_out[j].opt()])
                        # kt 0..7 (pair-rank 0 keys) go resident in f32r
                        kbf = kbf_pool.tile([P, 2, NQ], bf16, tag="kbf",
                                            name="kbf")
                        nc.sync.dma_start(kbf[:], cc_k_out[j][0])
                        nc.vector.tensor_copy(
                            k_sb[:, 2 * j:2 * j + 2, :], kbf[:])

                # ---------------- V proj + chunked AG_v -------------------
                for dh in range(2):
                    wvh = wvh_pool.tile([P, CC, 512], f32r, tag="wvh",
                                        name="wvh")
                    for cc in range(CC):
                        nc.sync.dma_start(
                            wvh[:, cc, :],
                            WvT_d[cc * P:(cc + 1) * P,
                                  dh * 512:(dh + 1) * 512])
                    for ks_ in range(CC):
                        pvt = pv_pool.tile([P, 512], f32, tag="pvt",
                                           name="pvt")
                        for cc in range(CC):
                            nc.tensor.matmul(
                                pvt[:],
                                xkv[:, cc, ks_ * P:(ks_ + 1) * P],
                                wvh[:, cc, :],
                                start=(cc == 0), stop=(cc == CC - 1))
                        vs = vstg_pool.tile([P, 512], bf16, tag="vs",
                                            name="vs")
                        nc.vector.tensor_copy(vs[:], pvt[:])
                        nc.sync.dma_start(
                            cc_v_in[ks_ // 2][(ks_ % 2) * P:
                                              (ks_ % 2 + 1) * P,
                                              dh * 512:(dh + 1) * 512],
                            vs[:])

                for j in range(4):
                    nc.gpsimd.collective_compute(
                        "AllGather", mybir.AluOpType.bypass,
                        replica_groups=RG,
                        ins=[cc_v_in[j].opt()], outs=[cc_v_out[j].opt()])
                    for p in range(2):
                        vbf = vbf_pool.tile([P, 2, C], bf16, tag="vbf",
                                            name="vbf")
                        nc.sync.dma_start(
                            vbf[:],
                            cc_v_out[j][p].rearrange(
                                "(i p) d -> p i d", p=P))
                        nc.vector.tensor_copy(
                            v_sb[:, p * CC + 2 * j:p * CC + 2 * j + 2, :],
                            vbf[:])

            # ---------------- Q proj (local tiles only) -------------------
            with tc.tile_pool(name="qp", bufs=1) as qp, \
                 tc.tile_pool(name="wq_pool", bufs=4) as wq_pool, \
                 tc.tile_pool(name="pq", bufs=2, space="PSUM") as pq_pool:
                xq = qp.tile([P, CC, NQ], f32r, tag="xq")
                for cc in range(CC):
                    nc.sync.dma_start(
                        xq[:, cc, :], xqT_d[cc * P:(cc + 1) * P, :])
                for dt in range(CC):
                    pqs = [pq_pool.tile([P, 512], f32, tag=f"pq{i}",
                                        name=f"pq{i}") for i in range(2)]
                    for cc in range(CC):
                        wq = wq_pool.tile([P, P], f32r, tag="wq", name="wq")
                        nc.sync.dma_start(
                            wq[:],
                            WqT_d[cc * P:(cc + 1) * P, dt * P:(dt + 1) * P])
                        for h in range(2):
                            nc.tensor.matmul(
                                pqs[h][:], wq[:],
                                xq[:, cc, h * 512:(h + 1) * 512],
                                start=(cc == 0), stop=(cc == CC - 1))
                    for h in range(2):
                        nc.vector.tensor_copy(
                            qT_sb[:, dt, h * 512:(h + 1) * 512], pqs[h][:])

            # ---------------- attention -----------------------------------
            with tc.tile_pool(name="attn", bufs=1) as attn, \
                 tc.tile_pool(name="msk_pool", bufs=2) as msk_pool, \
                 tc.tile_pool(name="ktf_pool", bufs=3) as ktf_pool, \
                 tc.tile_pool(name="ktb_pool", bufs=4) as ktb_pool, \
                 tc.tile_pool(name="out_pool", bufs=2) as out_pool, \
                 tc.tile_pool(name="rec_pool", bufs=2) as rec_pool, \
                 tc.tile_pool(name="ps", bufs=2, space="PSUM") as ps_pool, \
                 tc.tile_pool(name="pav", bufs=2, space="PSUM") as pav_pool, \
                 tc.tile_pool(name="pden", bufs=2, space="PSUM") as pden_pool:
                qposb = attn.tile([P, NQ], f32, tag="qposb")
                kpos = attn.tile([P, NKT], f32, tag="kpos")
                ones_f = attn.tile([P, 8], f32, tag="ones_f")
                ones = attn.tile([P, 8], f32r, tag="ones")
                # exp weights, shared by both chunks (free dim = in-chunk q)
                ex = attn.tile([P, NKT, 512], f32r, tag="ex")
                nc.sync.dma_start(qposb[:], qposb_d[:, :])
                nc.sync.dma_start(kpos[:], kpos_d[:, :])
                nc.vector.memset(ones_f[:], 1.0)
                nc.vector.tensor_copy(ones[:], ones_f[:])

                for c in range(2):
                    sl = slice(c * 512, (c + 1) * 512)
                    for kt in range(N_SC[c]):
                        if kt < CC:
                            lhs_kt = k_sb[:, :, kt * P:(kt + 1) * P]
                        else:
                            # pair-rank 1 keys: stream + widen from the AG
                            # bounce (4 d-chunk buffers make up one tile)
                            lk = kt - CC
                            ktf = ktf_pool.tile([P, CC, P], f32r, tag="ktf",
                                                name="ktf")
                            for j in range(4):
                                ktb = ktb_pool.tile([P, 2, P], bf16,
                                                    tag="ktb", name="ktb")
                                nc.sync.dma_start(
                                    ktb[:],
                                    cc_k_out[j][1][:, :,
                                                   lk * P:(lk + 1) * P])
                                nc.vector.tensor_copy(
                                    ktf[:, 2 * j:2 * j + 2, :], ktb[:])
                            lhs_kt = ktf[:, :, :]
                        ps = ps_pool.tile([P, 512], f32, tag="ps", name="ps")
                        for dc in range(CC):
                            nc.tensor.matmul(
                                ps[:], lhs_kt[:, dc, :], qT_sb[:, dc, sl],
                                start=(dc == 0), stop=(dc == CC - 1))
                        msk = msk_pool.tile([P, 512], f32, tag="msk",
                                            name="msk")
                        nc.vector.tensor_scalar(
                            msk[:], qposb[:, sl], kpos[:, kt:kt + 1],
                            None, op0=mybir.AluOpType.is_ge)
                        nc.scalar.activation(
                            ex[:, kt, :], ps[:],
                            mybir.ActivationFunctionType.Exp,
                            bias=0.0, scale=SCALE)
                        nc.vector.tensor_tensor(
                            ex[:, kt, :], ex[:, kt, :], msk[:],
                            op=mybir.AluOpType.mult)

                    for s in range(4 * c, 4 * c + 4):
                        pavs = [pav_pool.tile([P, 512], f32, tag=f"pav{i}",
                                              name=f"pav{i}")
                                for i in range(2)]
                        pden = pden_pool.tile([P, 8], f32, tag="pden",
                                              name="pden")
                        n = N_AV[s]
                        so = (s - 4 * c) * P
                        for kc in range(n):
                            lhs = ex[:, kc, so:so + P]
                            for dh in range(2):
                                nc.tensor.matmul(
                                    pavs[dh][:], lhs,
                                    v_sb[:, kc, dh * 512:(dh + 1) * 512],
                                    start=(kc == 0), stop=(kc == n - 1))
                            nc.tensor.matmul(
                                pden[:], lhs, ones[:],
                                start=(kc == 0), stop=(kc == n - 1))

                        rec = rec_pool.tile([P, 1], f32, tag="rec",
                                            name="rec")
                        nc.vector.reciprocal(rec[:], pden[:, 0:1])
                        for dh in range(2):
                            ot = out_pool.tile([P, 512], f32, tag="ot",
                                               name="ot")
                            nc.vector.tensor_scalar(
                                ot[:], pavs[dh][:], rec[:], None,
                                op0=mybir.AluOpType.mult)
                            nc.sync.dma_start(
                                out_d[s * P:(s + 1) * P,
                                      dh * 512:(dh + 1) * 512],
                                ot[:])

    nc.compile()
    return nc


def _get_compiled():
    global _COMPILED
    if _COMPILED is None:
        _COMPILED = _build_program()
    return _COMPILED


def _tf32_round(a):
    """Round fp32 to TF32 (10-bit mantissa), round-to-nearest-even."""
    u = a.view(np.uint32)
    r = ((u >> 13) + ((u >> 12) & 1)) << 13  # RNE-ish (ties up); fine here
    return r.astype(np.uint32).view(np.float32)


def _enable_ldw_opt():
    """walrus elides redundant back-to-back LDWEIGHTS with ldw-opt on; the
    repo default pins it off. Half our weight loads are consecutive dupes."""
    import concourse.bass_utils as _bu
    if getattr(_bu, "_ldw_patched", False):
        return
    orig = _bu.run_command

    def patched(argv, **kw):
        argv = ["--enable-ldw-opt=true" if a == "--enable-ldw-opt=false"
                else a for a in argv]
        return orig(argv, **kw)

    _bu.run_command = patched
    _bu._ldw_patched = True


def kernel(x, Wq, Wk, Wv):
    global LAST_RESULTS
    _enable_ldw_opt()
    from concourse.bass_utils import run_bass_kernel_spmd

    x = _tf32_round(np.ascontiguousarray(np.asarray(x, dtype=np.float32)))
    WqT = _tf32_round(np.ascontiguousarray(np.asarray(Wq, dtype=np.float32).T))
    WkT = _tf32_round(np.ascontiguousarray(np.asarray(Wk, dtype=np.float32).T))
    WvT = _tf32_round(np.ascontiguousarray(np.asarray(Wv, dtype=np.float32).T))

    kpos = (np.arange(NKT)[None, :] * P
            + np.arange(P)[:, None]).astype(np.float32)

    in_maps = []
    for core in range(NCORES):
        b, r = divmod(core, 2)
        xb_T = np.ascontiguousarray(x[b].T)            # [C, T]
        qcols = np.concatenate(
            [np.arange((2 * s + r) * P, (2 * s + r + 1) * P)
             for s in range(8)])
        xqT = np.ascontiguousarray(xb_T[:, qcols])
        xkvT = np.ascontiguousarray(xb_T[:, r * NQ:(r + 1) * NQ])
        qposb = np.ascontiguousarray(np.broadcast_to(
            qcols.astype(np.float32)[None, :], (P, NQ)))
        in_maps.append({
            "xqT": xqT, "xkvT": xkvT,
            "WqT": WqT, "WkT": WkT, "WvT": WvT,
            "qposb": qposb, "kpos": kpos,
        })

    nc = _get_compiled()
    res = run_bass_kernel_spmd(nc, in_maps, core_ids=list(range(NCORES)),
                               trace=TRACE)
    LAST_RESULTS = res

    out = np.empty((B, T, C), dtype=np.float32)
    for core in range(NCORES):
        b, r = divmod(core, 2)
        oc = res.results[core]["out"]                  # [NQ, C] local order
        for s in range(8):
            out[b, (2 * s + r) * P:(2 * s + r + 1) * P, :] = \
                oc[s * P:(s + 1) * P, :]
    return out
